# revision 7
# baseline (speedup 1.0000x reference)
"""Single-head attention (B=4, T=4096, D=1024, H=64, fp32 in/out) on 8 TRN2
NeuronCores.

Sharding: one core per (batch, T-half) pair -> 8 shards, no collectives.
Host pre-transposes and pre-casts every input so the device does zero input
transposes and minimal HBM traffic:
  xT      [8*1024, 512] bf16  t-blocked transpose of x[b] (query rows first)
  wqT     [1024, 64]    bf16  Wq^T
  wkvT    [1024, 128]   bf16  [Wk^T | Wv^T]
  maskT   [4096, 2048]  bf16  mask slice transposed to [s, t]
Each core returns its [2048, 64] f32 slice of the output.

Per-core pipeline:
  P phase: stream xT t-blocks (query blocks first, two DMA queues); fused
    k|v projection and q projection as N=512 matmuls accumulating over 8
    d-chunks in paired PSUM tiles. PSUM->SBUF copies on the (idle) Act
    engine; V' = [V | 1] via PE transposes + Pool copies.
  Attention (j-outer over 32 s-chunks, starts once query x-blocks are in):
    ST[j] = K[j] qT (4 matmuls N=512 into two [128,2,512] PSUM tiles),
    exp on Act ([128,1024] per instruction), mask multiply on DVE (2x bf16),
    PT fully resident in SBUF ([128, 32, 2048] bf16). PV matmuls for chunk
    j-2 are emitted adjacent to ST[j] so the PE stream stays dense (HAM
    stays un-throttled). PV accumulates 4 t-supertiles in one PSUM tile.
  Epilogue: 16 PE transposes back to [t, h], reciprocal of the ones-column,
    scale, store.
"""

import sys

if "/opt/trn_rl_repo" not in sys.path:
    sys.path.insert(0, "/opt/trn_rl_repo")

from contextlib import ExitStack

import numpy as np
import ml_dtypes

import concourse.bass as bass
import concourse.tile as tile
from concourse import bacc, mybir
from concourse.bass_utils import run_bass_kernel_spmd
from concourse.masks import make_identity

F32 = mybir.dt.float32
BF16 = mybir.dt.bfloat16

B, T, D, H = 4, 4096, 1024, 64
NCORES = 8
TQ = T // 2  # query rows per core

BF16NP = ml_dtypes.bfloat16


def build_attention_core(T=T, D=D, H=H, Tq=TQ):
    """Build the per-core Bass graph. Every core runs the same graph."""
    assert D % 128 == 0 and T % 1024 == 0 and Tq % 1024 == 0 and H == 64
    DC = D // 128          # d chunks (8)
    NS = T // 128          # s chunks (32)
    NTB = T // 512         # x t-blocks (8)
    NQB = Tq // 512        # query t-blocks (4)
    NSUP = Tq // 512       # t supertiles in attention (4)
    PVLAG = 2              # PV trails ST by this many s-chunks
    scale = 1.0 / float(np.sqrt(D))
    Exp = mybir.ActivationFunctionType.Exp

    nc = bacc.Bacc("TRN2", target_bir_lowering=False, debug=False,
                   num_devices=NCORES)
    xT_ext = nc.declare_dram_parameter("xt", [NTB * D, 512], BF16,
                                       isOutput=False)
    wqT_ext = nc.declare_dram_parameter("wqt", [D, H], BF16, isOutput=False)
    wkvT_ext = nc.declare_dram_parameter("wkvt", [D, 2 * H], BF16,
                                         isOutput=False)
    maskT_ext = nc.declare_dram_parameter("maskt", [T, Tq], BF16,
                                          isOutput=False)
    out_ext = nc.declare_dram_parameter("out", [Tq, H], F32, isOutput=True)

    with tile.TileContext(nc) as tc, ExitStack() as ctx:
        singles = ctx.enter_context(tc.tile_pool(name="singles", bufs=1))
        xin = ctx.enter_context(tc.tile_pool(name="xin", bufs=3))
        mpool = ctx.enter_context(tc.tile_pool(name="mpool", bufs=3))
        opool = ctx.enter_context(tc.tile_pool(name="opool", bufs=1))
        # PSUM: tag "p" [128,2,512] f32 x2 bufs (4 banks) shared by P phase,
        # ST tiles and epilogue; tag "pv" [128,4,512] f32 x1 (4 banks).
        psP = ctx.enter_context(tc.tile_pool(name="psP", bufs=2,
                                             space="PSUM"))
        psV = ctx.enter_context(tc.tile_pool(name="psV", bufs=1,
                                             space="PSUM"))

        ident_bf = singles.tile([128, 128], BF16)
        make_identity(nc, ident_bf)
        ident_f = singles.tile([128, 128], F32)
        make_identity(nc, ident_f)

        # ---- weights (already transposed on host) ----
        wqT_sb = singles.tile([128, DC, H], BF16)
        nc.scalar.dma_start(
            out=wqT_sb, in_=wqT_ext.rearrange("(c p) h -> p c h", p=128)
        )
        wkvT_sb = singles.tile([128, DC, 2 * H], BF16)
        nc.scalar.dma_start(
            out=wkvT_sb, in_=wkvT_ext.rearrange("(c p) h -> p c h", p=128)
        )

        # persistent activations
        kvT_sb = singles.tile([128, T], BF16)   # rows 0:64 kT, 64:128 vT
        qT_sb = singles.tile([H, Tq], BF16)
        Vp_sb = singles.tile([128, NS, H + 1], BF16)  # V' = [V | 1]
        PT_sb = singles.tile([128, NS, Tq], BF16)     # masked exp scores
        nc.gpsimd.memset(Vp_sb[:, :, H : H + 1], 1.0)

        # ---- P phase: stream x t-blocks -> k|v and q projections ----
        # Query blocks (0..3) first so attention can start before the full
        # x stream lands; alternate two hardware DMA queues.
        for tbp in range(NTB // 2):
            kv_ps = psP.tile([128, 2, 512], F32, tag="p")
            q_ps = None
            if tbp < NQB // 2:
                q_ps = psP.tile([128, 2, 512], F32, tag="p", name="q_ps")
            for half in range(2):
                tb = 2 * tbp + half
                x_sb = xin.tile([128, DC, 512], BF16, tag="x")
                dma_eng = nc.scalar if tb % 2 == 0 else nc.sync
                dma_eng.dma_start(
                    out=x_sb,
                    in_=xT_ext[tb * D : (tb + 1) * D, :].rearrange(
                        "(c p) t -> p c t", p=128
                    ),
                )
                for j in range(DC):
                    nc.tensor.matmul(
                        kv_ps[:, half, :],
                        wkvT_sb[:, j, :],
                        x_sb[:, j, :],
                        start=(j == 0),
                        stop=(j == DC - 1),
                    )
                if q_ps is not None:
                    for j in range(DC):
                        nc.tensor.matmul(
                            q_ps[0:H, half, :],
                            wqT_sb[:, j, :],
                            x_sb[:, j, :],
                            start=(j == 0),
                            stop=(j == DC - 1),
                        )
            nc.scalar.copy(
                kvT_sb[:, tbp * 1024 : (tbp + 1) * 1024],
                kv_ps.rearrange("p a b -> p (a b)"),
            )
            if q_ps is not None:
                nc.scalar.copy(
                    qT_sb[:, tbp * 1024 : (tbp + 1) * 1024],
                    q_ps[0:H].rearrange("p a b -> p (a b)"),
                )
            # V natural layout for the 8 s-chunks of this t-block pair
            vt_ps = psP.tile([128, 8, H], BF16, tag="p")
            for jj in range(8):
                s0 = tbp * 1024 + jj * 128
                nc.tensor.transpose(
                    vt_ps[:, jj, :],
                    kvT_sb[H : 2 * H, s0 : s0 + 128],
                    ident_bf[H : 2 * H, H : 2 * H],
                )
            nc.vector.tensor_copy(
                Vp_sb[:, tbp * 8 : (tbp + 1) * 8, 0:H], vt_ps
            )

        # ---- attention: ST/exp/mask with PV trailing by PVLAG chunks ----
        pv_ps = psV.tile([128, NSUP, 512], F32, tag="pv")

        def pv_step(j):
            for ts in range(NSUP):
                nc.tensor.matmul(
                    pv_ps[0 : H + 1, ts, :],
                    Vp_sb[:, j, :],
                    PT_sb[:, j, ts * 512 : (ts + 1) * 512],
                    start=(j == 0),
                    stop=(j == NS - 1),
                )

        for j in range(NS):
            m_sb = mpool.tile([128, Tq], BF16, tag="m")
            nc.sync.dma_start(
                out=m_sb, in_=maskT_ext[j * 128 : (j + 1) * 128, :]
            )
            for hh in range(2):
                st_ps = psP.tile([128, 2, 512], F32, tag="p")
                for ts in range(2):
                    t0 = (2 * hh + ts) * 512
                    nc.tensor.matmul(
                        st_ps[:, ts, :],
                        kvT_sb[0:H, j * 128 : (j + 1) * 128],
                        qT_sb[:, t0 : t0 + 512],
                    )
                nc.scalar.activation(
                    PT_sb[:, j, hh * 1024 : (hh + 1) * 1024],
                    st_ps.rearrange("p a b -> p (a b)"),
                    Exp,
                    scale=scale,
                )
                nc.vector.tensor_mul(
                    PT_sb[:, j, hh * 1024 : (hh + 1) * 1024],
                    PT_sb[:, j, hh * 1024 : (hh + 1) * 1024],
                    m_sb[:, hh * 1024 : (hh + 1) * 1024],
                )
            if j >= PVLAG:
                pv_step(j - PVLAG)
        for j in range(NS - PVLAG, NS):
            pv_step(j)

        # ---- epilogue: transpose back to [t, h], normalize, store ----
        oT_sb = opool.tile([H + 1, Tq], F32, tag="oT")
        nc.vector.tensor_copy(
            oT_sb, pv_ps[0 : H + 1].rearrange("p a b -> p (a b)")
        )
        for half in range(2):
            ep_ps = psP.tile([128, 2, 512], F32, tag="p")
            for gg in range(8):
                c0 = half * 1024 + gg * 128
                bank, sl = gg // 4, gg % 4
                nc.tensor.transpose(
                    ep_ps[:, bank, sl * (H + 1) : (sl + 1) * (H + 1)],
                    oT_sb[:, c0 : c0 + 128],
                    ident_f[0 : H + 1, 0 : H + 1],
                )
            o_sb = opool.tile([128, 8, H + 1], F32, tag="o")
            nc.vector.tensor_copy(
                o_sb.rearrange("p (a g) b -> p a g b", a=2),
                ep_ps[:, :, 0 : 4 * (H + 1)].rearrange(
                    "p a (g b) -> p a g b", b=H + 1
                ),
            )
            r_sb = opool.tile([128, 8, 1], F32, tag="r")
            nc.vector.reciprocal(r_sb, o_sb[:, :, H : H + 1])
            for gg in range(8):
                nc.vector.tensor_scalar_mul(
                    o_sb[:, gg, 0:H], o_sb[:, gg, 0:H], r_sb[:, gg, :]
                )
            rows = out_ext[half * 1024 : (half + 1) * 1024, :].rearrange(
                "(g p) h -> p g h", p=128
            )
            nc.sync.dma_start(out=rows, in_=o_sb[:, :, 0:H])
    nc.compile()
    return nc


_NC_CACHE = {}


def _get_nc(shape_key):
    if shape_key not in _NC_CACHE:
        T_, D_, H_, Tq_ = shape_key
        _NC_CACHE[shape_key] = build_attention_core(T=T_, D=D_, H=H_, Tq=Tq_)
    return _NC_CACHE[shape_key]


def _prep_inputs(x, Wq, Wk, Wv, mask):
    """Host-side shard + transpose + cast. Core c -> (batch c//2, half c%2).
    The x rows of the core's query half come first; mask columns get the
    same permutation so key order matches the permuted x rows."""
    x = np.ascontiguousarray(x, dtype=np.float32)
    mask = np.ascontiguousarray(mask, dtype=np.int32)
    Bv, Tv, Dv = x.shape
    Tq = Tv // 2
    ntb = Tv // 512

    wqT = np.ascontiguousarray(np.asarray(Wq, dtype=np.float32).T).astype(
        BF16NP
    )
    wkvT = np.concatenate(
        [np.asarray(Wk, dtype=np.float32).T, np.asarray(Wv, np.float32).T],
        axis=1,
    ).astype(BF16NP)

    def block_xt(xb):
        # [T, D] -> transpose -> t-blocked [(tb d), 512]
        xt = xb.T.astype(BF16NP)  # [D, T]
        return np.ascontiguousarray(
            xt.reshape(Dv, ntb, 512).transpose(1, 0, 2).reshape(ntb * Dv, 512)
        )

    # mask is shared across batches: only two variants (one per half)
    m0 = mask[0, 0:Tq, :]  # [t, s] for half 0
    m1 = np.concatenate([mask[0, Tq:, Tq:], mask[0, Tq:, :Tq]], axis=1)
    maskT0 = np.ascontiguousarray(m0.T.astype(BF16NP))
    maskT1 = np.ascontiguousarray(m1.T.astype(BF16NP))

    in_maps = []
    for c in range(NCORES):
        b, half = c // 2, c % 2
        if half == 0:
            xc = x[b]
            mT = maskT0
        else:
            xc = np.concatenate([x[b, Tq:], x[b, :Tq]], axis=0)
            mT = maskT1
        in_maps.append(
            {
                "xt": block_xt(xc),
                "wqt": wqT,
                "wkvt": wkvT,
                "maskt": mT,
            }
        )
    return in_maps


def kernel(x, Wq, Wk, Wv, mask, _trace=False):
    x = np.asarray(x)
    Bv, Tv, Dv = x.shape
    Hv = np.asarray(Wq).shape[0]
    Tq = Tv // 2
    nc = _get_nc((Tv, Dv, Hv, Tq))
    in_maps = _prep_inputs(
        np.asarray(x), np.asarray(Wq), np.asarray(Wk), np.asarray(Wv),
        np.asarray(mask),
    )
    res = run_bass_kernel_spmd(
        nc, in_maps, core_ids=list(range(NCORES)), trace=_trace
    )
    out = np.empty((Bv, Tv, Hv), dtype=np.float32)
    for c in range(NCORES):
        b, half = c // 2, c % 2
        out[b, half * Tq : (half + 1) * Tq] = res.results[c]["out"]
    if _trace:
        kernel.last_results = res
    return out


# revision 11
# speedup vs baseline: 1.4238x; 1.4238x over previous
"""Single-head attention (B=4, T=4096, D=1024, H=64, fp32 in/out) on 8 TRN2
NeuronCores.

Sharding: one core per (batch, T-half) pair -> 8 shards, no collectives.
Host pre-transposes and pre-casts every input so the device does zero input
transposes and minimal HBM traffic:
  xT      [8*1024, 512] bf16  t-blocked transpose of x[b] (query rows first)
  wqT     [1024, 64]    bf16  Wq^T
  wkvT    [1024, 128]   bf16  [Wk^T | Wv^T]
  maskT   [4096, 2048]  bf16  mask slice transposed to [s, t]
Each core returns its [2048, 64] f32 slice of the output.

Per-core pipeline:
  P phase: stream xT t-blocks (query blocks first, two DMA queues); fused
    k|v projection and q projection as N=512 matmuls accumulating over 8
    d-chunks in paired PSUM tiles. PSUM->SBUF copies on the (idle) Act
    engine; V' = [V | 1] via PE transposes + Pool copies.
  Attention (j-outer over 32 s-chunks, starts once query x-blocks are in):
    ST[j] = K[j] qT (4 matmuls N=512 into two [128,2,512] PSUM tiles),
    exp on Act ([128,1024] per instruction), mask multiply on DVE (2x bf16),
    PT fully resident in SBUF ([128, 32, 2048] bf16). PV matmuls for chunk
    j-2 are emitted adjacent to ST[j] so the PE stream stays dense (HAM
    stays un-throttled). PV accumulates 4 t-supertiles in one PSUM tile.
  Epilogue: 16 PE transposes back to [t, h], reciprocal of the ones-column,
    scale, store.
"""

import sys

if "/opt/trn_rl_repo" not in sys.path:
    sys.path.insert(0, "/opt/trn_rl_repo")

from contextlib import ExitStack

import numpy as np
import ml_dtypes

import concourse.bass as bass
import concourse.tile as tile
from concourse import bacc, mybir
from concourse.bass_utils import run_bass_kernel_spmd
from concourse.masks import make_identity

F32 = mybir.dt.float32
BF16 = mybir.dt.bfloat16

B, T, D, H = 4, 4096, 1024, 64
NCORES = 8
TQ = T // 2  # query rows per core

BF16NP = ml_dtypes.bfloat16


def build_attention_core(T=T, D=D, H=H, Tq=TQ):
    """Build the per-core Bass graph. Every core runs the same graph."""
    assert D % 128 == 0 and T % 1024 == 0 and Tq % 1024 == 0 and H == 64
    DC = D // 128          # d chunks (8)
    NS = T // 128          # s chunks (32)
    NTB = T // 512         # x t-blocks (8)
    NQB = Tq // 512        # query t-blocks (4)
    NSUP = Tq // 512       # t supertiles in attention (4)
    PVLAG = 2              # PV trails ST by this many s-chunks
    scale = 1.0 / float(np.sqrt(D))
    Exp = mybir.ActivationFunctionType.Exp

    nc = bacc.Bacc("TRN2", target_bir_lowering=False, debug=False,
                   num_devices=NCORES)
    xT_ext = nc.declare_dram_parameter("xt", [NTB * D, 512], BF16,
                                       isOutput=False)
    wqT_ext = nc.declare_dram_parameter("wqt", [D, H], BF16, isOutput=False)
    wkvT_ext = nc.declare_dram_parameter("wkvt", [D, 2 * H], BF16,
                                         isOutput=False)
    maskT_ext = nc.declare_dram_parameter("maskt", [T, Tq], BF16,
                                          isOutput=False)
    out_ext = nc.declare_dram_parameter("out", [Tq, H], F32, isOutput=True)

    with tile.TileContext(nc) as tc, ExitStack() as ctx:
        singles = ctx.enter_context(tc.tile_pool(name="singles", bufs=1))
        xin = ctx.enter_context(tc.tile_pool(name="xin", bufs=3))
        mpool = ctx.enter_context(tc.tile_pool(name="mpool", bufs=3))
        opool = ctx.enter_context(tc.tile_pool(name="opool", bufs=1))
        # PSUM: tag "p" [128,2,512] f32 x2 bufs (4 banks) shared by P phase,
        # ST tiles and epilogue; tag "pv" [128,4,512] f32 x1 (4 banks).
        psP = ctx.enter_context(tc.tile_pool(name="psP", bufs=2,
                                             space="PSUM"))
        psV = ctx.enter_context(tc.tile_pool(name="psV", bufs=1,
                                             space="PSUM"))

        ident_bf = singles.tile([128, 128], BF16)
        make_identity(nc, ident_bf)
        ident_f = singles.tile([128, 128], F32)
        make_identity(nc, ident_f)

        # ---- weights (already transposed on host) ----
        wqT_sb = singles.tile([128, DC, H], BF16)
        nc.scalar.dma_start(
            out=wqT_sb, in_=wqT_ext.rearrange("(c p) h -> p c h", p=128)
        )
        wkvT_sb = singles.tile([128, DC, 2 * H], BF16)
        nc.scalar.dma_start(
            out=wkvT_sb, in_=wkvT_ext.rearrange("(c p) h -> p c h", p=128)
        )

        # persistent activations. qT and V' are zero-padded to the full 128
        # partition/column width so attention matmuls light up the whole PE
        # array (HAM un-throttles only under full-array activity): the vT
        # rows of kvT meet zero q rows, and V' columns 65:128 are zero.
        kvT_sb = singles.tile([128, T], BF16)   # rows 0:64 kT, 64:128 vT
        qT_sb = singles.tile([128, Tq], BF16)   # rows 64:128 zero
        Vp_sb = singles.tile([128, NS, 128], BF16)  # V' = [V | 1 | 0pad]
        PT_sb = singles.tile([128, NS, Tq], BF16)   # masked exp scores
        nc.gpsimd.memset(qT_sb[H : 2 * H, :], 0.0)
        nc.gpsimd.memset(Vp_sb[:, :, H + 1 : 128], 0.0)
        nc.gpsimd.memset(Vp_sb[:, :, H : H + 1], 1.0)

        # ---- P phase: stream x t-blocks -> k|v and q projections ----
        # Query blocks (0..3) first so attention can start before the full
        # x stream lands; alternate two hardware DMA queues.
        for tbp in range(NTB // 2):
            kv_ps = psP.tile([128, 2, 512], F32, tag="p")
            q_ps = None
            if tbp < NQB // 2:
                q_ps = psP.tile([128, 2, 512], F32, tag="p", name="q_ps")
            for half in range(2):
                tb = 2 * tbp + half
                x_sb = xin.tile([128, DC, 512], BF16, tag="x")
                dma_eng = nc.scalar if tb % 2 == 0 else nc.sync
                dma_eng.dma_start(
                    out=x_sb,
                    in_=xT_ext[tb * D : (tb + 1) * D, :].rearrange(
                        "(c p) t -> p c t", p=128
                    ),
                )
                for j in range(DC):
                    nc.tensor.matmul(
                        kv_ps[:, half, :],
                        wkvT_sb[:, j, :],
                        x_sb[:, j, :],
                        start=(j == 0),
                        stop=(j == DC - 1),
                    )
                if q_ps is not None:
                    for j in range(DC):
                        nc.tensor.matmul(
                            q_ps[0:H, half, :],
                            wqT_sb[:, j, :],
                            x_sb[:, j, :],
                            start=(j == 0),
                            stop=(j == DC - 1),
                        )
            nc.scalar.copy(
                kvT_sb[:, tbp * 1024 : (tbp + 1) * 1024],
                kv_ps.rearrange("p a b -> p (a b)"),
            )
            if q_ps is not None:
                nc.scalar.copy(
                    qT_sb[0:H, tbp * 1024 : (tbp + 1) * 1024],
                    q_ps[0:H].rearrange("p a b -> p (a b)"),
                )
            # V natural layout for the 8 s-chunks of this t-block pair
            vt_ps = psP.tile([128, 8, H], BF16, tag="p")
            for jj in range(8):
                s0 = tbp * 1024 + jj * 128
                nc.tensor.transpose(
                    vt_ps[:, jj, :],
                    kvT_sb[H : 2 * H, s0 : s0 + 128],
                    ident_bf[H : 2 * H, H : 2 * H],
                )
            nc.vector.tensor_copy(
                Vp_sb[:, tbp * 8 : (tbp + 1) * 8, 0:H], vt_ps
            )

        # ---- attention: ST/exp/mask with PV trailing by PVLAG chunks ----
        pv_ps = psV.tile([128, NSUP, 512], F32, tag="pv")

        def pv_step(j):
            for ts in range(NSUP):
                nc.tensor.matmul(
                    pv_ps[:, ts, :],
                    Vp_sb[:, j, :],
                    PT_sb[:, j, ts * 512 : (ts + 1) * 512],
                    start=(j == 0),
                    stop=(j == NS - 1),
                )

        for j in range(NS):
            m_sb = mpool.tile([128, Tq], BF16, tag="m")
            nc.sync.dma_start(
                out=m_sb, in_=maskT_ext[j * 128 : (j + 1) * 128, :]
            )
            for hh in range(2):
                st_ps = psP.tile([128, 2, 512], F32, tag="p")
                for ts in range(2):
                    t0 = (2 * hh + ts) * 512
                    nc.tensor.matmul(
                        st_ps[:, ts, :],
                        kvT_sb[:, j * 128 : (j + 1) * 128],
                        qT_sb[:, t0 : t0 + 512],
                    )
                nc.scalar.activation(
                    PT_sb[:, j, hh * 1024 : (hh + 1) * 1024],
                    st_ps.rearrange("p a b -> p (a b)"),
                    Exp,
                    scale=scale,
                )
                nc.vector.tensor_mul(
                    PT_sb[:, j, hh * 1024 : (hh + 1) * 1024],
                    PT_sb[:, j, hh * 1024 : (hh + 1) * 1024],
                    m_sb[:, hh * 1024 : (hh + 1) * 1024],
                )
            if j >= PVLAG:
                pv_step(j - PVLAG)
        for j in range(NS - PVLAG, NS):
            pv_step(j)

        # ---- epilogue: transpose back to [t, h], normalize, store ----
        oT_sb = opool.tile([H + 1, Tq], F32, tag="oT")
        nc.vector.tensor_copy(
            oT_sb, pv_ps[0 : H + 1].rearrange("p a b -> p (a b)")
        )
        for half in range(2):
            ep_ps = psP.tile([128, 2, 512], F32, tag="p")
            for gg in range(8):
                c0 = half * 1024 + gg * 128
                bank, sl = gg // 4, gg % 4
                nc.tensor.transpose(
                    ep_ps[:, bank, sl * (H + 1) : (sl + 1) * (H + 1)],
                    oT_sb[:, c0 : c0 + 128],
                    ident_f[0 : H + 1, 0 : H + 1],
                )
            o_sb = opool.tile([128, 8, H + 1], F32, tag="o")
            nc.vector.tensor_copy(
                o_sb.rearrange("p (a g) b -> p a g b", a=2),
                ep_ps[:, :, 0 : 4 * (H + 1)].rearrange(
                    "p a (g b) -> p a g b", b=H + 1
                ),
            )
            r_sb = opool.tile([128, 8, 1], F32, tag="r")
            nc.vector.reciprocal(r_sb, o_sb[:, :, H : H + 1])
            for gg in range(8):
                nc.vector.tensor_scalar_mul(
                    o_sb[:, gg, 0:H], o_sb[:, gg, 0:H], r_sb[:, gg, :]
                )
            rows = out_ext[half * 1024 : (half + 1) * 1024, :].rearrange(
                "(g p) h -> p g h", p=128
            )
            nc.sync.dma_start(out=rows, in_=o_sb[:, :, 0:H])
    nc.compile()
    return nc


_NC_CACHE = {}


def _get_nc(shape_key):
    if shape_key not in _NC_CACHE:
        T_, D_, H_, Tq_ = shape_key
        _NC_CACHE[shape_key] = build_attention_core(T=T_, D=D_, H=H_, Tq=Tq_)
    return _NC_CACHE[shape_key]


def _prep_inputs(x, Wq, Wk, Wv, mask):
    """Host-side shard + transpose + cast. Core c -> (batch c//2, half c%2).
    The x rows of the core's query half come first; mask columns get the
    same permutation so key order matches the permuted x rows."""
    x = np.ascontiguousarray(x, dtype=np.float32)
    mask = np.ascontiguousarray(mask, dtype=np.int32)
    Bv, Tv, Dv = x.shape
    Tq = Tv // 2
    ntb = Tv // 512

    wqT = np.ascontiguousarray(np.asarray(Wq, dtype=np.float32).T).astype(
        BF16NP
    )
    wkvT = np.concatenate(
        [np.asarray(Wk, dtype=np.float32).T, np.asarray(Wv, np.float32).T],
        axis=1,
    ).astype(BF16NP)

    def block_xt(xb):
        # [T, D] -> transpose -> t-blocked [(tb d), 512]
        xt = xb.T.astype(BF16NP)  # [D, T]
        return np.ascontiguousarray(
            xt.reshape(Dv, ntb, 512).transpose(1, 0, 2).reshape(ntb * Dv, 512)
        )

    # mask is shared across batches: only two variants (one per half)
    m0 = mask[0, 0:Tq, :]  # [t, s] for half 0
    m1 = np.concatenate([mask[0, Tq:, Tq:], mask[0, Tq:, :Tq]], axis=1)
    maskT0 = np.ascontiguousarray(m0.T.astype(BF16NP))
    maskT1 = np.ascontiguousarray(m1.T.astype(BF16NP))

    in_maps = []
    for c in range(NCORES):
        b, half = c // 2, c % 2
        if half == 0:
            xc = x[b]
            mT = maskT0
        else:
            xc = np.concatenate([x[b, Tq:], x[b, :Tq]], axis=0)
            mT = maskT1
        in_maps.append(
            {
                "xt": block_xt(xc),
                "wqt": wqT,
                "wkvt": wkvT,
                "maskt": mT,
            }
        )
    return in_maps


def kernel(x, Wq, Wk, Wv, mask, _trace=False):
    x = np.asarray(x)
    Bv, Tv, Dv = x.shape
    Hv = np.asarray(Wq).shape[0]
    Tq = Tv // 2
    nc = _get_nc((Tv, Dv, Hv, Tq))
    in_maps = _prep_inputs(
        np.asarray(x), np.asarray(Wq), np.asarray(Wk), np.asarray(Wv),
        np.asarray(mask),
    )
    res = run_bass_kernel_spmd(
        nc, in_maps, core_ids=list(range(NCORES)), trace=_trace
    )
    out = np.empty((Bv, Tv, Hv), dtype=np.float32)
    for c in range(NCORES):
        b, half = c // 2, c % 2
        out[b, half * Tq : (half + 1) * Tq] = res.results[c]["out"]
    if _trace:
        kernel.last_results = res
    return out
